# revision 4
# baseline (speedup 1.0000x reference)
"""DCN-v1 (dense_mlp) Trainium2 kernel.

Strategy (8 NeuronCores, SPMD):
  - Data-parallel over batch: 16384 rows -> 2048 per core.
  - Embedding lookups via batched SWDGE dma_gather (InstDMAGatherAnt):
    tables packed 4 vocab rows per 256B super-row so super-indices fit
    int16; 1024 lookups per instruction, round-robined over the 4 SWDGE
    queues (the drain rate is ~8.8ns/descriptor/queue, so 4 queues ~4x).
    Sub-row select + multi-hot sum-pool fused as one DVE mask-multiply +
    one strided reduce per (field, group).
  - CrossNet collapsed algebraically: the whole cross stack + final-layer
    cross dot reduce to ONE small matmul P = x0 @ [w_0..w_3, lin_w[:448]]
    plus a tiny scalar recurrence (cross_b == 0).
  - MLP in bf16 with fp32 PSUM accumulation; ReLU+bias fused on ScalarE.

Performance notes (measured, 8 cores):
  - Baseline with per-row indirect DMA gathers: 2.04 ms (1408 gather
    instructions/core, each one 128-row SWDGE instruction ~1.45us).
  - dma_gather batches 1024 rows per instruction (176/core). The gather
    stream is descriptor-rate-bound at ~30GB/s per SWDGE queue; 4 queues
    sustain ~115GB/s/core -> ~390us for the 46MB/core of 256B fetches.
    Compute (PE ~90us, DVE ~115us) hides under the gather stream.
  - >1536 idxs per instruction overflows the per-queue descriptor ring
    (2048 entries) and wedges the exec unit; 1024 is the safe point.
  - Fetch size does not matter (128B descriptors time identically), so
    fp8 tables or tighter packing buy nothing.
"""

import os
import sys

import numpy as np
import ml_dtypes

for _p in ("/opt/trn_rl_repo", os.path.expanduser("~/.axon_site/_ro/trn_rl_repo")):
    if os.path.isdir(_p) and _p not in sys.path:
        sys.path.append(_p)

B = 16384
N_CORES = 8
BL = B // N_CORES  # 2048 rows per core
DENSE = 64
N_OH, N_MH, HIST = 8, 4, 20
VOCAB = 100000
EMB = 32
IN_DIM = 448
HID = [1024, 512, 256]
CHUNK = 128  # samples per chunk (partition dim)
NCH = BL // CHUNK  # 16 chunks per core
NBLK = 512  # samples per matmul n-block
KS = [128, 128, 128, 64]  # k-tile sizes over the 448-dim input features
BF16 = ml_dtypes.bfloat16

SROWS = VOCAB // 4  # 25000 super-rows of 4 vocab rows (256B bf16)
SES = 4 * EMB  # 128 elems per super-row
GK = 1024  # lookups per dma_gather instruction
# one-hot: per field, 2 gathers of 1024 (8 chunks each)
OH_G = 2
# multi-hot: per field, groups of 2 chunks (40 slots = 2*20), 5 gathers each
MH_CPG = 2  # chunks per group
MH_GRP = NCH // MH_CPG  # 8 groups
MH_SLOTS = MH_CPG * HIST  # 40 slots per group
MH_KPG = MH_SLOTS * CHUNK // GK  # 5 gathers per group


def _build_program(c_consts, sig_bias):
    """Build the SPMD Bass/Tile program. c_consts[l] = C_l . w_l (fp32),
    sig_bias = C_4 . lin_w[:448] + lin_b."""
    from contextlib import ExitStack

    import concourse.bass as bass
    import concourse.tile as tile
    from concourse import bacc, mybir
    from concourse.masks import make_identity

    dt = mybir.dt
    AF = mybir.ActivationFunctionType
    n_chunks_per_nb = NBLK // CHUNK  # 4
    n_nb = BL // NBLK  # 4

    nc = bacc.Bacc(num_swdge_queues=4)
    dense_d = nc.dram_tensor("dense", [128, NCH, DENSE], dt.bfloat16, kind="ExternalInput")
    ohtab_d = [
        nc.dram_tensor(f"ohtab{f}", [SROWS, SES], dt.bfloat16, kind="ExternalInput")
        for f in range(N_OH)
    ]
    mhtab_d = [
        nc.dram_tensor(f"mhtab{f}", [SROWS, SES], dt.bfloat16, kind="ExternalInput")
        for f in range(N_MH)
    ]
    ohidx_d = nc.dram_tensor(
        "ohidx", [128, N_OH, OH_G, GK // 16], dt.int16, kind="ExternalInput"
    )
    mhidx_d = nc.dram_tensor(
        "mhidx", [128, N_MH, MH_GRP, MH_KPG, GK // 16], dt.int16, kind="ExternalInput"
    )
    ohmsk_d = nc.dram_tensor(
        "ohmsk", [128, N_OH, NCH, 4], dt.bfloat16, kind="ExternalInput"
    )
    mhmsk_d = nc.dram_tensor(
        "mhmsk", [128, N_MH, NCH * HIST, 4], dt.bfloat16, kind="ExternalInput"
    )
    w1_d = nc.dram_tensor("w1p", [128, 4, 1024], dt.bfloat16, kind="ExternalInput")
    w2_d = nc.dram_tensor("w2p", [128, 8, 512], dt.bfloat16, kind="ExternalInput")
    w3_d = nc.dram_tensor("w3p", [128, 4, 256], dt.bfloat16, kind="ExternalInput")
    wsm_d = nc.dram_tensor("wsm", [128, 22], dt.bfloat16, kind="ExternalInput")
    bias_d = nc.dram_tensor("biasp", [128, 14], dt.float32, kind="ExternalInput")
    out_d = nc.dram_tensor("out", [128, NCH], dt.float32, kind="ExternalOutput")

    with ExitStack() as ctx:
        tc = ctx.enter_context(tile.TileContext(nc))
        wp = ctx.enter_context(tc.tile_pool(name="weights", bufs=1))
        ohgp = ctx.enter_context(tc.tile_pool(name="ohg", bufs=1))
        mhgp = ctx.enter_context(tc.tile_pool(name="mhg", bufs=3))
        tp_mh = ctx.enter_context(tc.tile_pool(name="tmh", bufs=2))
        tp_oh = ctx.enter_context(tc.tile_pool(name="toh", bufs=2))
        x0p = ctx.enter_context(tc.tile_pool(name="x0", bufs=3))
        xtp = ctx.enter_context(tc.tile_pool(name="xt", bufs=2))
        hp = ctx.enter_context(tc.tile_pool(name="h", bufs=2))
        recp = ctx.enter_context(tc.tile_pool(name="rec", bufs=2))
        ps_mm = ctx.enter_context(tc.tile_pool(name="psmm", bufs=3, space="PSUM"))
        ps_tr = ctx.enter_context(tc.tile_pool(name="pstr", bufs=2, space="PSUM"))
        ps_sm = ctx.enter_context(tc.tile_pool(name="pssm", bufs=1, space="PSUM"))
        ps_q2 = ctx.enter_context(tc.tile_pool(name="psq2", bufs=2, space="PSUM"))

        # --- resident weights / indices / masks ---
        w1_sb = wp.tile([128, 4, 1024], dt.bfloat16)
        nc.sync.dma_start(w1_sb[:], w1_d[:])
        w2_sb = wp.tile([128, 8, 512], dt.bfloat16)
        nc.sync.dma_start(w2_sb[:], w2_d[:])
        w3_sb = wp.tile([128, 4, 256], dt.bfloat16)
        nc.sync.dma_start(w3_sb[:], w3_d[:])
        wsm_sb = wp.tile([128, 22], dt.bfloat16)
        nc.sync.dma_start(wsm_sb[:], wsm_d[:])
        bias_sb = wp.tile([128, 14], dt.float32)
        nc.sync.dma_start(bias_sb[:], bias_d[:])
        ident = wp.tile([128, 128], dt.bfloat16)
        make_identity(nc, ident[:])
        dense_sb = wp.tile([128, NCH, DENSE], dt.bfloat16)
        nc.sync.dma_start(dense_sb[:], dense_d[:])
        ohidx_sb = wp.tile([128, N_OH, OH_G, GK // 16], dt.int16)
        nc.sync.dma_start(ohidx_sb[:], ohidx_d[:])
        mhidx_sb = wp.tile([128, N_MH, MH_GRP, MH_KPG, GK // 16], dt.int16)
        nc.sync.dma_start(mhidx_sb[:], mhidx_d[:])
        ohmsk_sb = wp.tile([128, N_OH, NCH, 4], dt.bfloat16)
        nc.sync.dma_start(ohmsk_sb[:], ohmsk_d[:])
        mhmsk_sb = wp.tile([128, N_MH, NCH * HIST, 4], dt.bfloat16)
        nc.sync.dma_start(mhmsk_sb[:], mhmsk_d[:])
        # selected embeddings, whole core: [p, chunk, field, 32]
        ohsel = wp.tile([128, NCH, N_OH, EMB], dt.bfloat16)
        mhsel = wp.tile([128, NCH, N_MH, EMB], dt.bfloat16)
        out_sb = wp.tile([128, NCH], dt.float32)

        qn = [0]

        def next_q():
            q = qn[0] % 4
            qn[0] += 1
            return q

        # ---- one-hot: gather super-rows, mask-select sub-row ----
        for f in range(N_OH):
            ohg = ohgp.tile([128, NCH, SES], dt.bfloat16, tag=f"ohg{f}")
            for g in range(OH_G):
                nc.gpsimd.dma_gather(
                    ohg[:, g * 8 : (g + 1) * 8, :],
                    ohtab_d[f][:],
                    ohidx_sb[:, f, g],
                    GK,
                    GK,
                    SES,
                    queue_num=next_q(),
                )
            t = tp_oh.tile([128, NCH, 4, EMB], dt.bfloat16, tag="toh")
            nc.vector.tensor_tensor(
                out=t[:],
                in0=ohg[:].rearrange("p c (q e) -> p c q e", q=4),
                in1=ohmsk_sb[:, f][:, :, :, None].broadcast_to([128, NCH, 4, EMB]),
                op=mybir.AluOpType.mult,
            )
            with nc.allow_low_precision(
                reason="one-of-4 mask select: single nonzero bf16 term, exact"
            ):
                nc.vector.tensor_reduce(
                    out=ohsel[:, :, f, :],
                    in_=t[:].rearrange("p c q e -> p c e q"),
                    axis=mybir.AxisListType.X,
                    op=mybir.AluOpType.add,
                )

        # ---- multi-hot: gather, fused mask-select + sum-pool ----
        for g in range(MH_GRP):
            for f in range(N_MH):
                mhg = mhgp.tile([128, MH_SLOTS, SES], dt.bfloat16, tag="mhg")
                for k in range(MH_KPG):
                    nc.gpsimd.dma_gather(
                        mhg[:, k * 8 : (k + 1) * 8, :],
                        mhtab_d[f][:],
                        mhidx_sb[:, f, g, k],
                        GK,
                        GK,
                        SES,
                        queue_num=next_q(),
                    )
                t = tp_mh.tile([128, MH_SLOTS, 4, EMB], dt.bfloat16, tag="tmh")
                sl = slice(g * MH_SLOTS, (g + 1) * MH_SLOTS)
                nc.vector.tensor_tensor(
                    out=t[:],
                    in0=mhg[:].rearrange("p t (q e) -> p t q e", q=4),
                    in1=mhmsk_sb[:, f, sl][:, :, :, None].broadcast_to(
                        [128, MH_SLOTS, 4, EMB]
                    ),
                    op=mybir.AluOpType.mult,
                )
                mp32 = tp_oh.tile([128, MH_CPG, EMB], dt.float32, tag="mp32")
                nc.vector.tensor_reduce(
                    out=mp32[:],
                    in_=t[:].rearrange("p (c h) q e -> p c e (h q)", h=HIST),
                    axis=mybir.AxisListType.X,
                    op=mybir.AluOpType.add,
                )
                nc.vector.tensor_copy(
                    mhsel[:, g * MH_CPG : (g + 1) * MH_CPG, f, :], mp32[:]
                )

        # ---- per n-block: assemble x0^T, cross projections, MLP ----
        for nb in range(n_nb):
            x0T = xtp.tile([128, 4, NBLK], dt.bfloat16, tag="x0T")
            lgq1 = recp.tile([128, 4], dt.float32, tag="lgq1")
            for cc in range(n_chunks_per_nb):
                c = nb * n_chunks_per_nb + cc
                cs = slice(cc * CHUNK, (cc + 1) * CHUNK)

                x0c = x0p.tile([128, 512], dt.bfloat16, tag="x0c")
                nc.vector.memset(x0c[:, 448:512], 0.0)
                nc.vector.tensor_copy(x0c[:, 0:DENSE], dense_sb[:, c, :])
                nc.vector.tensor_copy(
                    x0c[:, DENSE : DENSE + N_OH * EMB],
                    ohsel[:, c].rearrange("p f e -> p (f e)"),
                )
                nc.vector.tensor_copy(
                    x0c[:, DENSE + N_OH * EMB : IN_DIM],
                    mhsel[:, c].rearrange("p f e -> p (f e)"),
                )

                # transpose the 128-sample chunk to feature-major
                tp = ps_tr.tile([128, 4, 128], dt.bfloat16, tag="trps")
                for j in range(4):
                    nc.tensor.transpose(
                        tp[:, j : j + 1, :],
                        x0c[:, j * 128 : (j + 1) * 128],
                        ident[:],
                    )
                nc.vector.tensor_copy(x0T[:, :, cs], tp[:])

                # cross projections: pn[s, l] = x0 . w_l (l<4), pn[s,4] = x0 . lin_w[:448]
                pn = ps_sm.tile([128, 5], dt.float32, tag="pn")
                for j in range(4):
                    nc.tensor.matmul(
                        pn[:],
                        x0T[0 : KS[j], j : j + 1, cs],
                        wsm_sb[0 : KS[j], j * 5 : j * 5 + 5],
                        start=(j == 0),
                        stop=(j == 3),
                    )
                # logit cross part: prod(1+p_l) * q1  (cross_b == 0)
                pp1 = recp.tile([128, 4], dt.float32, tag="pp1")
                nc.vector.tensor_scalar_add(pp1[:], pn[:, 0:4], 1.0)
                m01 = recp.tile([128, 1], dt.float32, tag="m01")
                nc.vector.tensor_mul(m01[:], pp1[:, 0:1], pp1[:, 1:2])
                m23 = recp.tile([128, 1], dt.float32, tag="m23")
                nc.vector.tensor_mul(m23[:], pp1[:, 2:3], pp1[:, 3:4])
                a4 = recp.tile([128, 1], dt.float32, tag="a4")
                nc.vector.tensor_mul(a4[:], m01[:], m23[:])
                nc.vector.tensor_mul(lgq1[:, cc : cc + 1], a4[:], pn[:, 4:5])

            # ---- deep net ----
            h1 = hp.tile([128, 8, NBLK], dt.bfloat16, tag="h1")
            for m in range(8):
                ps = ps_mm.tile([128, NBLK], dt.float32, tag="mm")
                for j in range(4):
                    nc.tensor.matmul(
                        ps[:],
                        w1_sb[0 : KS[j], j : j + 1, m * 128 : (m + 1) * 128],
                        x0T[0 : KS[j], j : j + 1, :],
                        start=(j == 0),
                        stop=(j == 3),
                    )
                nc.scalar.activation(
                    h1[:, m : m + 1, :], ps[:], AF.Relu, bias=bias_sb[:, m : m + 1]
                )
            h2 = hp.tile([128, 4, NBLK], dt.bfloat16, tag="h2")
            for m in range(4):
                ps = ps_mm.tile([128, NBLK], dt.float32, tag="mm")
                for j in range(8):
                    nc.tensor.matmul(
                        ps[:],
                        w2_sb[:, j : j + 1, m * 128 : (m + 1) * 128],
                        h1[:, j : j + 1, :],
                        start=(j == 0),
                        stop=(j == 7),
                    )
                nc.scalar.activation(
                    h2[:, m : m + 1, :], ps[:], AF.Relu, bias=bias_sb[:, 8 + m : 9 + m]
                )
            h3 = hp.tile([128, 2, NBLK], dt.bfloat16, tag="h3")
            for m in range(2):
                ps = ps_mm.tile([128, NBLK], dt.float32, tag="mm")
                for j in range(4):
                    nc.tensor.matmul(
                        ps[:],
                        w3_sb[:, j : j + 1, m * 128 : (m + 1) * 128],
                        h2[:, j : j + 1, :],
                        start=(j == 0),
                        stop=(j == 3),
                    )
                nc.scalar.activation(
                    h3[:, m : m + 1, :], ps[:], AF.Relu, bias=bias_sb[:, 12 + m : 13 + m]
                )

            # ---- final: logit = prod(1+p)*q1 + h3.lin_w_bot + sig_bias ----
            for cc in range(n_chunks_per_nb):
                c = nb * n_chunks_per_nb + cc
                cs = slice(cc * CHUNK, (cc + 1) * CHUNK)
                q2n = ps_q2.tile([128, 1], dt.float32, tag="q2n")
                for j in range(2):
                    nc.tensor.matmul(
                        q2n[:],
                        h3[:, j : j + 1, cs],
                        wsm_sb[:, 20 + j : 21 + j],
                        start=(j == 0),
                        stop=(j == 1),
                    )
                lg2 = recp.tile([128, 1], dt.float32, tag="lg2")
                nc.vector.tensor_add(lg2[:], lgq1[:, cc : cc + 1], q2n[:])
                nc.scalar.activation(
                    out_sb[:, c : c + 1], lg2[:], AF.Sigmoid, bias=float(sig_bias)
                )

        nc.sync.dma_start(out_d[:], out_sb[:])

    nc.compile()
    return nc


def _wrap16(vals):
    """int16 index layout for dma_gather: logical idx j lives at
    [j % 16, j // 16], replicated across the 8 gpsimd cores."""
    return np.tile(vals.reshape(-1, 16).T, (8, 1))


def _prep_inputs(
    dense_x,
    one_hot_x,
    multi_hot_x,
    one_hot_emb,
    multi_hot_emb,
    cross_w,
    cross_b,
    W1,
    b1,
    W2,
    b2,
    W3,
    b3,
    lin_w,
    lin_b,
):
    dense_bf = np.ascontiguousarray(dense_x, dtype=np.float32).astype(BF16)
    oh_emb = np.ascontiguousarray(one_hot_emb, dtype=np.float32).astype(BF16)
    mh_emb = np.ascontiguousarray(multi_hot_emb, dtype=np.float32).astype(BF16)

    oh_idx = np.asarray(one_hot_x, np.int64)  # (B, 8)
    mh_idx = np.asarray(multi_hot_x, np.int64)  # (B, 4, 20)

    def pack_k(Wmat, out_cols):
        p = np.zeros((128, 4, out_cols), np.float32)
        for j in range(4):
            p[0 : KS[j], j, :] = Wmat[j * 128 : j * 128 + KS[j], :]
        return p.astype(BF16)

    w1p = pack_k(np.asarray(W1, np.float32), 1024)
    w2p = (
        np.asarray(W2, np.float32).reshape(8, 128, 512).transpose(1, 0, 2).copy()
    ).astype(BF16)
    w3p = (
        np.asarray(W3, np.float32).reshape(4, 128, 256).transpose(1, 0, 2).copy()
    ).astype(BF16)
    lw = np.asarray(lin_w, np.float32)[:, 0]
    cwq = pack_k(
        np.concatenate([np.asarray(cross_w, np.float32).T, lw[:IN_DIM, None]], 1), 5
    )
    wsm = np.zeros((128, 22), np.float32)
    wsm[:, 0:20] = cwq.astype(np.float32).reshape(128, 20)
    wsm[:, 20:22] = lw[IN_DIM:].reshape(2, 128).T
    wsm = wsm.astype(BF16)
    biasp = np.concatenate(
        [
            np.asarray(b1, np.float32).reshape(8, 128).T,
            np.asarray(b2, np.float32).reshape(4, 128).T,
            np.asarray(b3, np.float32).reshape(2, 128).T,
        ],
        axis=1,
    ).copy()

    # cross-net constants: C_0 = 0, C_{l+1} = C_l + b_l ; c_l = C_l . w_l
    cb = np.asarray(cross_b, np.float64)
    cwf = np.asarray(cross_w, np.float64)
    C = np.zeros(IN_DIM, np.float64)
    c_consts = []
    for l in range(4):
        c_consts.append(float(C @ cwf[l]))
        C = C + cb[l]
    sig_bias = float(C @ np.asarray(lw[:IN_DIM], np.float64)) + float(
        np.asarray(lin_b, np.float64).reshape(-1)[0]
    )
    if any(abs(c) > 1e-30 for c in c_consts):
        raise NotImplementedError(
            "cross_b != 0 would need the general recurrence; this model's "
            "setup always has cross_b == 0"
        )

    shared = {
        "w1p": w1p,
        "w2p": w2p,
        "w3p": w3p,
        "wsm": wsm,
        "biasp": biasp,
    }
    for f in range(N_OH):
        shared[f"ohtab{f}"] = np.ascontiguousarray(oh_emb[f].reshape(SROWS, SES))
    for f in range(N_MH):
        shared[f"mhtab{f}"] = np.ascontiguousarray(mh_emb[f].reshape(SROWS, SES))

    eye4 = np.eye(4, dtype=np.float32).astype(BF16)

    in_maps = []
    for core in range(N_CORES):
        rs = slice(core * BL, (core + 1) * BL)
        m = dict(shared)
        m["dense"] = np.ascontiguousarray(
            dense_bf[rs].reshape(NCH, 128, DENSE).transpose(1, 0, 2)
        )
        ohc = oh_idx[rs]  # (2048, 8)
        mhc = mh_idx[rs]  # (2048, 4, 20)

        # one-hot idxs: gather (f, g) covers chunks [8g, 8g+8);
        # j = c_local*128 + s -> sample (8g + c_local)*128 + s
        ohi = np.empty((128, N_OH, OH_G, GK // 16), np.int16)
        for f in range(N_OH):
            for g in range(OH_G):
                sm = ohc[g * GK : (g + 1) * GK, f]  # samples in j order
                ohi[:, f, g, :] = _wrap16((sm >> 2).astype(np.int16))
        m["ohidx"] = np.ascontiguousarray(ohi)

        # multi-hot idxs: gather (f, grp, k) covers slots t = k*8 + j//128
        # within the group; global slot T = grp*40 + t maps to
        # (chunk, h) = (grp*2 + T//20 - grp*2 ... ) -> c = T//20, h = T%20;
        # sample = c*128 + s.
        mhi = np.empty((128, N_MH, MH_GRP, MH_KPG, GK // 16), np.int16)
        for f in range(N_MH):
            for g in range(MH_GRP):
                for k in range(MH_KPG):
                    t = g * MH_SLOTS + k * 8 + np.arange(8)  # global slots
                    c, h = t // HIST, t % HIST
                    # vals[j] for j = tl*128 + s
                    vals = mhc[(c[:, None] * 128 + np.arange(128)[None, :]), f, h[:, None]]
                    mhi[:, f, g, k, :] = _wrap16((vals.reshape(-1) >> 2).astype(np.int16))
        m["mhidx"] = np.ascontiguousarray(mhi)

        # masks: one-hot [p, f, c, q]; multi-hot [p, f, c*20+h, q]
        m["ohmsk"] = np.ascontiguousarray(
            eye4[ohc.reshape(NCH, 128, N_OH) & 3].transpose(1, 2, 0, 3)
        )  # (128, 8, 16, 4)
        mq = mhc.reshape(NCH, 128, N_MH, HIST) & 3  # (16, 128, 4, 20)
        m["mhmsk"] = np.ascontiguousarray(
            eye4[mq].transpose(1, 2, 0, 3, 4).reshape(128, N_MH, NCH * HIST, 4)
        )
        in_maps.append(m)
    return in_maps, c_consts, sig_bias


def _run(inputs, trace=False):
    from concourse.bass_utils import run_bass_kernel_spmd

    in_maps, c_consts, sig_bias = _prep_inputs(**inputs)
    nc = _build_program(c_consts, sig_bias)
    res = run_bass_kernel_spmd(nc, in_maps, core_ids=list(range(N_CORES)), trace=trace)
    outs = [
        res.results[c]["out"].reshape(128, NCH).T.reshape(BL) for c in range(N_CORES)
    ]
    full = np.concatenate(outs).reshape(B, 1).astype(np.float32)
    return full, res


def kernel(**inputs):
    full, _ = _run(inputs, trace=False)
    return full


# revision 5
# speedup vs baseline: 1.1338x; 1.1338x over previous
"""DCN-v1 (dense_mlp) Trainium2 kernel.

Strategy (8 NeuronCores, SPMD):
  - Data-parallel over batch: 16384 rows -> 2048 per core.
  - Embedding lookups via batched SWDGE dma_gather (InstDMAGatherAnt):
    tables packed 4 vocab rows per 256B super-row so super-indices fit
    int16; 1024 lookups per instruction, round-robined over the 4 SWDGE
    queues (the drain rate is ~8.8ns/descriptor/queue, so 4 queues ~4x).
    Sub-row select + multi-hot sum-pool fused as one DVE mask-multiply +
    one strided reduce per (field, group).
  - CrossNet collapsed algebraically: the whole cross stack + final-layer
    cross dot reduce to ONE small matmul P = x0 @ [w_0..w_3, lin_w[:448]]
    plus a tiny scalar recurrence (cross_b == 0).
  - MLP in bf16 with fp32 PSUM accumulation; ReLU+bias fused on ScalarE.

Performance notes (measured, 8 cores):
  - Baseline with per-row indirect DMA gathers: 2.04 ms (1408 gather
    instructions/core, each one 128-row SWDGE instruction ~1.45us).
  - dma_gather batches 1024 rows per instruction (176/core). The gather
    stream is descriptor-rate-bound at ~30GB/s per SWDGE queue; 4 queues
    sustain ~115GB/s/core -> ~390us for the 46MB/core of 256B fetches.
    Compute (PE ~90us, DVE ~115us) hides under the gather stream.
  - >1536 idxs per instruction overflows the per-queue descriptor ring
    (2048 entries) and wedges the exec unit; 1024 is the safe point.
  - Fetch size does not matter (128B descriptors time identically), so
    fp8 tables or tighter packing buy nothing.
"""

import os
import sys

import numpy as np
import ml_dtypes

for _p in ("/opt/trn_rl_repo", os.path.expanduser("~/.axon_site/_ro/trn_rl_repo")):
    if os.path.isdir(_p) and _p not in sys.path:
        sys.path.append(_p)

B = 16384
N_CORES = 8
BL = B // N_CORES  # 2048 rows per core
DENSE = 64
N_OH, N_MH, HIST = 8, 4, 20
VOCAB = 100000
EMB = 32
IN_DIM = 448
HID = [1024, 512, 256]
CHUNK = 128  # samples per chunk (partition dim)
NCH = BL // CHUNK  # 16 chunks per core
NBLK = 512  # samples per matmul n-block
KS = [128, 128, 128, 64]  # k-tile sizes over the 448-dim input features
BF16 = ml_dtypes.bfloat16

SROWS = VOCAB // 4  # 25000 super-rows of 4 vocab rows (256B bf16)
SES = 4 * EMB  # 128 elems per super-row
GK = 1024  # lookups per dma_gather instruction
# one-hot: per field, 2 gathers of 1024 (8 chunks each)
OH_G = 2
# multi-hot: per field, groups of 2 chunks (40 slots = 2*20), 5 gathers each
MH_CPG = 2  # chunks per group
MH_GRP = NCH // MH_CPG  # 8 groups
MH_SLOTS = MH_CPG * HIST  # 40 slots per group
MH_KPG = MH_SLOTS * CHUNK // GK  # 5 gathers per group


def _build_program(c_consts, sig_bias):
    """Build the SPMD Bass/Tile program. c_consts[l] = C_l . w_l (fp32),
    sig_bias = C_4 . lin_w[:448] + lin_b."""
    from contextlib import ExitStack

    import concourse.bass as bass
    import concourse.tile as tile
    from concourse import bacc, mybir
    from concourse.masks import make_identity

    dt = mybir.dt
    AF = mybir.ActivationFunctionType
    n_chunks_per_nb = NBLK // CHUNK  # 4
    n_nb = BL // NBLK  # 4

    nc = bacc.Bacc(num_swdge_queues=4)
    dense_d = nc.dram_tensor("dense", [128, NCH, DENSE], dt.bfloat16, kind="ExternalInput")
    ohtab_d = [
        nc.dram_tensor(f"ohtab{f}", [SROWS, SES], dt.bfloat16, kind="ExternalInput")
        for f in range(N_OH)
    ]
    mhtab_d = [
        nc.dram_tensor(f"mhtab{f}", [SROWS, SES], dt.bfloat16, kind="ExternalInput")
        for f in range(N_MH)
    ]
    ohidx_d = nc.dram_tensor(
        "ohidx", [128, N_OH, OH_G, GK // 16], dt.int16, kind="ExternalInput"
    )
    mhidx_d = nc.dram_tensor(
        "mhidx", [128, N_MH, MH_GRP, MH_KPG, GK // 16], dt.int16, kind="ExternalInput"
    )
    ohmsk_d = nc.dram_tensor(
        "ohmsk", [128, N_OH, NCH, 4], dt.bfloat16, kind="ExternalInput"
    )
    mhmsk_d = nc.dram_tensor(
        "mhmsk", [128, N_MH, NCH * HIST, 4], dt.bfloat16, kind="ExternalInput"
    )
    w1_d = nc.dram_tensor("w1p", [128, 4, 1024], dt.bfloat16, kind="ExternalInput")
    w2_d = nc.dram_tensor("w2p", [128, 8, 512], dt.bfloat16, kind="ExternalInput")
    w3_d = nc.dram_tensor("w3p", [128, 4, 256], dt.bfloat16, kind="ExternalInput")
    wsm_d = nc.dram_tensor("wsm", [128, 22], dt.bfloat16, kind="ExternalInput")
    bias_d = nc.dram_tensor("biasp", [128, 14], dt.float32, kind="ExternalInput")
    out_d = nc.dram_tensor("out", [128, NCH], dt.float32, kind="ExternalOutput")

    with ExitStack() as ctx:
        tc = ctx.enter_context(tile.TileContext(nc))
        wp = ctx.enter_context(tc.tile_pool(name="weights", bufs=1))
        ohgp = ctx.enter_context(tc.tile_pool(name="ohg", bufs=1))
        mhgp = ctx.enter_context(tc.tile_pool(name="mhg", bufs=3))
        tp_mh = ctx.enter_context(tc.tile_pool(name="tmh", bufs=2))
        tp_oh = ctx.enter_context(tc.tile_pool(name="toh", bufs=2))
        x0p = ctx.enter_context(tc.tile_pool(name="x0", bufs=3))
        xtp = ctx.enter_context(tc.tile_pool(name="xt", bufs=2))
        hp = ctx.enter_context(tc.tile_pool(name="h", bufs=2))
        recp = ctx.enter_context(tc.tile_pool(name="rec", bufs=2))
        ps_mm = ctx.enter_context(tc.tile_pool(name="psmm", bufs=3, space="PSUM"))
        ps_tr = ctx.enter_context(tc.tile_pool(name="pstr", bufs=2, space="PSUM"))
        ps_sm = ctx.enter_context(tc.tile_pool(name="pssm", bufs=1, space="PSUM"))
        ps_q2 = ctx.enter_context(tc.tile_pool(name="psq2", bufs=2, space="PSUM"))

        # --- resident weights / indices / masks ---
        w1_sb = wp.tile([128, 4, 1024], dt.bfloat16)
        nc.sync.dma_start(w1_sb[:], w1_d[:])
        w2_sb = wp.tile([128, 8, 512], dt.bfloat16)
        nc.sync.dma_start(w2_sb[:], w2_d[:])
        w3_sb = wp.tile([128, 4, 256], dt.bfloat16)
        nc.sync.dma_start(w3_sb[:], w3_d[:])
        wsm_sb = wp.tile([128, 22], dt.bfloat16)
        nc.sync.dma_start(wsm_sb[:], wsm_d[:])
        bias_sb = wp.tile([128, 14], dt.float32)
        nc.sync.dma_start(bias_sb[:], bias_d[:])
        ident = wp.tile([128, 128], dt.bfloat16)
        make_identity(nc, ident[:])
        dense_sb = wp.tile([128, NCH, DENSE], dt.bfloat16)
        nc.sync.dma_start(dense_sb[:], dense_d[:])
        ohidx_sb = wp.tile([128, N_OH, OH_G, GK // 16], dt.int16)
        nc.sync.dma_start(ohidx_sb[:], ohidx_d[:])
        mhidx_sb = wp.tile([128, N_MH, MH_GRP, MH_KPG, GK // 16], dt.int16)
        nc.sync.dma_start(mhidx_sb[:], mhidx_d[:])
        ohmsk_sb = wp.tile([128, N_OH, NCH, 4], dt.bfloat16)
        nc.sync.dma_start(ohmsk_sb[:], ohmsk_d[:])
        mhmsk_sb = wp.tile([128, N_MH, NCH * HIST, 4], dt.bfloat16)
        nc.sync.dma_start(mhmsk_sb[:], mhmsk_d[:])
        # selected embeddings, whole core: [p, chunk, field, 32]
        ohsel = wp.tile([128, NCH, N_OH, EMB], dt.bfloat16)
        mhsel32 = wp.tile([128, NCH, N_MH, EMB], dt.float32)
        mhsel = wp.tile([128, NCH, N_MH, EMB], dt.bfloat16)
        out_sb = wp.tile([128, NCH], dt.float32)

        qn = [0]

        def next_q():
            q = qn[0] % 4
            qn[0] += 1
            return q

        # ---- one-hot: gather super-rows, mask-select sub-row ----
        for f in range(N_OH):
            ohg = ohgp.tile([128, NCH, SES], dt.bfloat16, tag=f"ohg{f}")
            for g in range(OH_G):
                nc.gpsimd.dma_gather(
                    ohg[:, g * 8 : (g + 1) * 8, :],
                    ohtab_d[f][:],
                    ohidx_sb[:, f, g],
                    GK,
                    GK,
                    SES,
                    queue_num=next_q(),
                )
            t = tp_oh.tile([128, NCH, EMB, 4], dt.bfloat16, tag="toh")
            nc.vector.tensor_tensor(
                out=t[:],
                in0=ohg[:].rearrange("p c (e q) -> p c e q", q=4),
                in1=ohmsk_sb[:, f][:, :, None, :].broadcast_to([128, NCH, EMB, 4]),
                op=mybir.AluOpType.mult,
            )
            with nc.allow_low_precision(
                reason="one-of-4 mask select: single nonzero bf16 term, exact"
            ):
                nc.vector.tensor_reduce(
                    out=ohsel[:, :, f, :],
                    in_=t[:],
                    axis=mybir.AxisListType.X,
                    op=mybir.AluOpType.add,
                )

        # ---- multi-hot: gather, fused mask-select + sum-pool ----
        for g in range(MH_GRP):
            for f in range(N_MH):
                mhg = mhgp.tile([128, MH_SLOTS, SES], dt.bfloat16, tag="mhg")
                for k in range(MH_KPG):
                    nc.gpsimd.dma_gather(
                        mhg[:, k * 8 : (k + 1) * 8, :],
                        mhtab_d[f][:],
                        mhidx_sb[:, f, g, k],
                        GK,
                        GK,
                        SES,
                        queue_num=next_q(),
                    )
                t = tp_mh.tile([128, MH_SLOTS, EMB, 4], dt.bfloat16, tag="tmh")
                sl = slice(g * MH_SLOTS, (g + 1) * MH_SLOTS)
                nc.vector.tensor_tensor(
                    out=t[:],
                    in0=mhg[:].rearrange("p t (e q) -> p t e q", q=4),
                    in1=mhmsk_sb[:, f, sl][:, :, None, :].broadcast_to(
                        [128, MH_SLOTS, EMB, 4]
                    ),
                    op=mybir.AluOpType.mult,
                )
                s1 = tp_oh.tile([128, MH_SLOTS, EMB], dt.bfloat16, tag="s1")
                with nc.allow_low_precision(
                    reason="one-of-4 mask select: single nonzero bf16 term, exact"
                ):
                    nc.vector.tensor_reduce(
                        out=s1[:],
                        in_=t[:],
                        axis=mybir.AxisListType.X,
                        op=mybir.AluOpType.add,
                    )
                nc.vector.tensor_reduce(
                    out=mhsel32[:, g * MH_CPG : (g + 1) * MH_CPG, f, :],
                    in_=s1[:].rearrange("p (c h) e -> p c e h", h=HIST),
                    axis=mybir.AxisListType.X,
                    op=mybir.AluOpType.add,
                )

        nc.vector.tensor_copy(mhsel[:], mhsel32[:])

        # ---- per n-block: assemble x0^T, cross projections, MLP ----
        for nb in range(n_nb):
            x0T = xtp.tile([128, 4, NBLK], dt.bfloat16, tag="x0T")
            lgq1 = recp.tile([128, 4], dt.float32, tag="lgq1")
            for cc in range(n_chunks_per_nb):
                c = nb * n_chunks_per_nb + cc
                cs = slice(cc * CHUNK, (cc + 1) * CHUNK)

                x0c = x0p.tile([128, 512], dt.bfloat16, tag="x0c")
                nc.vector.memset(x0c[:, 448:512], 0.0)
                nc.vector.tensor_copy(x0c[:, 0:DENSE], dense_sb[:, c, :])
                nc.vector.tensor_copy(
                    x0c[:, DENSE : DENSE + N_OH * EMB],
                    ohsel[:, c].rearrange("p f e -> p (f e)"),
                )
                nc.vector.tensor_copy(
                    x0c[:, DENSE + N_OH * EMB : IN_DIM],
                    mhsel[:, c].rearrange("p f e -> p (f e)"),
                )

                # transpose the 128-sample chunk to feature-major
                tp = ps_tr.tile([128, 4, 128], dt.bfloat16, tag="trps")
                for j in range(4):
                    nc.tensor.transpose(
                        tp[:, j : j + 1, :],
                        x0c[:, j * 128 : (j + 1) * 128],
                        ident[:],
                    )
                nc.vector.tensor_copy(x0T[:, :, cs], tp[:])

                # cross projections: pn[s, l] = x0 . w_l (l<4), pn[s,4] = x0 . lin_w[:448]
                pn = ps_sm.tile([128, 5], dt.float32, tag="pn")
                for j in range(4):
                    nc.tensor.matmul(
                        pn[:],
                        x0T[0 : KS[j], j : j + 1, cs],
                        wsm_sb[0 : KS[j], j * 5 : j * 5 + 5],
                        start=(j == 0),
                        stop=(j == 3),
                    )
                # logit cross part: prod(1+p_l) * q1  (cross_b == 0)
                pp1 = recp.tile([128, 4], dt.float32, tag="pp1")
                nc.vector.tensor_scalar_add(pp1[:], pn[:, 0:4], 1.0)
                m01 = recp.tile([128, 1], dt.float32, tag="m01")
                nc.vector.tensor_mul(m01[:], pp1[:, 0:1], pp1[:, 1:2])
                m23 = recp.tile([128, 1], dt.float32, tag="m23")
                nc.vector.tensor_mul(m23[:], pp1[:, 2:3], pp1[:, 3:4])
                a4 = recp.tile([128, 1], dt.float32, tag="a4")
                nc.vector.tensor_mul(a4[:], m01[:], m23[:])
                nc.vector.tensor_mul(lgq1[:, cc : cc + 1], a4[:], pn[:, 4:5])

            # ---- deep net ----
            h1 = hp.tile([128, 8, NBLK], dt.bfloat16, tag="h1")
            for m in range(8):
                ps = ps_mm.tile([128, NBLK], dt.float32, tag="mm")
                for j in range(4):
                    nc.tensor.matmul(
                        ps[:],
                        w1_sb[0 : KS[j], j : j + 1, m * 128 : (m + 1) * 128],
                        x0T[0 : KS[j], j : j + 1, :],
                        start=(j == 0),
                        stop=(j == 3),
                    )
                nc.scalar.activation(
                    h1[:, m : m + 1, :], ps[:], AF.Relu, bias=bias_sb[:, m : m + 1]
                )
            h2 = hp.tile([128, 4, NBLK], dt.bfloat16, tag="h2")
            for m in range(4):
                ps = ps_mm.tile([128, NBLK], dt.float32, tag="mm")
                for j in range(8):
                    nc.tensor.matmul(
                        ps[:],
                        w2_sb[:, j : j + 1, m * 128 : (m + 1) * 128],
                        h1[:, j : j + 1, :],
                        start=(j == 0),
                        stop=(j == 7),
                    )
                nc.scalar.activation(
                    h2[:, m : m + 1, :], ps[:], AF.Relu, bias=bias_sb[:, 8 + m : 9 + m]
                )
            h3 = hp.tile([128, 2, NBLK], dt.bfloat16, tag="h3")
            for m in range(2):
                ps = ps_mm.tile([128, NBLK], dt.float32, tag="mm")
                for j in range(4):
                    nc.tensor.matmul(
                        ps[:],
                        w3_sb[:, j : j + 1, m * 128 : (m + 1) * 128],
                        h2[:, j : j + 1, :],
                        start=(j == 0),
                        stop=(j == 3),
                    )
                nc.scalar.activation(
                    h3[:, m : m + 1, :], ps[:], AF.Relu, bias=bias_sb[:, 12 + m : 13 + m]
                )

            # ---- final: logit = prod(1+p)*q1 + h3.lin_w_bot + sig_bias ----
            for cc in range(n_chunks_per_nb):
                c = nb * n_chunks_per_nb + cc
                cs = slice(cc * CHUNK, (cc + 1) * CHUNK)
                q2n = ps_q2.tile([128, 1], dt.float32, tag="q2n")
                for j in range(2):
                    nc.tensor.matmul(
                        q2n[:],
                        h3[:, j : j + 1, cs],
                        wsm_sb[:, 20 + j : 21 + j],
                        start=(j == 0),
                        stop=(j == 1),
                    )
                lg2 = recp.tile([128, 1], dt.float32, tag="lg2")
                nc.vector.tensor_add(lg2[:], lgq1[:, cc : cc + 1], q2n[:])
                nc.scalar.activation(
                    out_sb[:, c : c + 1], lg2[:], AF.Sigmoid, bias=float(sig_bias)
                )

        nc.sync.dma_start(out_d[:], out_sb[:])

    nc.compile()
    return nc


def _wrap16(vals):
    """int16 index layout for dma_gather: logical idx j lives at
    [j % 16, j // 16], replicated across the 8 gpsimd cores."""
    return np.tile(vals.reshape(-1, 16).T, (8, 1))


def _prep_inputs(
    dense_x,
    one_hot_x,
    multi_hot_x,
    one_hot_emb,
    multi_hot_emb,
    cross_w,
    cross_b,
    W1,
    b1,
    W2,
    b2,
    W3,
    b3,
    lin_w,
    lin_b,
):
    dense_bf = np.ascontiguousarray(dense_x, dtype=np.float32).astype(BF16)
    oh_emb = np.ascontiguousarray(one_hot_emb, dtype=np.float32).astype(BF16)
    mh_emb = np.ascontiguousarray(multi_hot_emb, dtype=np.float32).astype(BF16)

    oh_idx = np.asarray(one_hot_x, np.int64)  # (B, 8)
    mh_idx = np.asarray(multi_hot_x, np.int64)  # (B, 4, 20)

    def pack_k(Wmat, out_cols):
        p = np.zeros((128, 4, out_cols), np.float32)
        for j in range(4):
            p[0 : KS[j], j, :] = Wmat[j * 128 : j * 128 + KS[j], :]
        return p.astype(BF16)

    w1p = pack_k(np.asarray(W1, np.float32), 1024)
    w2p = (
        np.asarray(W2, np.float32).reshape(8, 128, 512).transpose(1, 0, 2).copy()
    ).astype(BF16)
    w3p = (
        np.asarray(W3, np.float32).reshape(4, 128, 256).transpose(1, 0, 2).copy()
    ).astype(BF16)
    lw = np.asarray(lin_w, np.float32)[:, 0]
    cwq = pack_k(
        np.concatenate([np.asarray(cross_w, np.float32).T, lw[:IN_DIM, None]], 1), 5
    )
    wsm = np.zeros((128, 22), np.float32)
    wsm[:, 0:20] = cwq.astype(np.float32).reshape(128, 20)
    wsm[:, 20:22] = lw[IN_DIM:].reshape(2, 128).T
    wsm = wsm.astype(BF16)
    biasp = np.concatenate(
        [
            np.asarray(b1, np.float32).reshape(8, 128).T,
            np.asarray(b2, np.float32).reshape(4, 128).T,
            np.asarray(b3, np.float32).reshape(2, 128).T,
        ],
        axis=1,
    ).copy()

    # cross-net constants: C_0 = 0, C_{l+1} = C_l + b_l ; c_l = C_l . w_l
    cb = np.asarray(cross_b, np.float64)
    cwf = np.asarray(cross_w, np.float64)
    C = np.zeros(IN_DIM, np.float64)
    c_consts = []
    for l in range(4):
        c_consts.append(float(C @ cwf[l]))
        C = C + cb[l]
    sig_bias = float(C @ np.asarray(lw[:IN_DIM], np.float64)) + float(
        np.asarray(lin_b, np.float64).reshape(-1)[0]
    )
    if any(abs(c) > 1e-30 for c in c_consts):
        raise NotImplementedError(
            "cross_b != 0 would need the general recurrence; this model's "
            "setup always has cross_b == 0"
        )

    shared = {
        "w1p": w1p,
        "w2p": w2p,
        "w3p": w3p,
        "wsm": wsm,
        "biasp": biasp,
    }
    for f in range(N_OH):
        shared[f"ohtab{f}"] = np.ascontiguousarray(
            oh_emb[f].reshape(SROWS, 4, EMB).transpose(0, 2, 1).reshape(SROWS, SES)
        )
    for f in range(N_MH):
        shared[f"mhtab{f}"] = np.ascontiguousarray(
            mh_emb[f].reshape(SROWS, 4, EMB).transpose(0, 2, 1).reshape(SROWS, SES)
        )

    eye4 = np.eye(4, dtype=np.float32).astype(BF16)

    in_maps = []
    for core in range(N_CORES):
        rs = slice(core * BL, (core + 1) * BL)
        m = dict(shared)
        m["dense"] = np.ascontiguousarray(
            dense_bf[rs].reshape(NCH, 128, DENSE).transpose(1, 0, 2)
        )
        ohc = oh_idx[rs]  # (2048, 8)
        mhc = mh_idx[rs]  # (2048, 4, 20)

        # one-hot idxs: gather (f, g) covers chunks [8g, 8g+8);
        # j = c_local*128 + s -> sample (8g + c_local)*128 + s
        ohi = np.empty((128, N_OH, OH_G, GK // 16), np.int16)
        for f in range(N_OH):
            for g in range(OH_G):
                sm = ohc[g * GK : (g + 1) * GK, f]  # samples in j order
                ohi[:, f, g, :] = _wrap16((sm >> 2).astype(np.int16))
        m["ohidx"] = np.ascontiguousarray(ohi)

        # multi-hot idxs: gather (f, grp, k) covers slots t = k*8 + j//128
        # within the group; global slot T = grp*40 + t maps to
        # (chunk, h) = (grp*2 + T//20 - grp*2 ... ) -> c = T//20, h = T%20;
        # sample = c*128 + s.
        mhi = np.empty((128, N_MH, MH_GRP, MH_KPG, GK // 16), np.int16)
        for f in range(N_MH):
            for g in range(MH_GRP):
                for k in range(MH_KPG):
                    t = g * MH_SLOTS + k * 8 + np.arange(8)  # global slots
                    c, h = t // HIST, t % HIST
                    # vals[j] for j = tl*128 + s
                    vals = mhc[(c[:, None] * 128 + np.arange(128)[None, :]), f, h[:, None]]
                    mhi[:, f, g, k, :] = _wrap16((vals.reshape(-1) >> 2).astype(np.int16))
        m["mhidx"] = np.ascontiguousarray(mhi)

        # masks: one-hot [p, f, c, q]; multi-hot [p, f, c*20+h, q]
        m["ohmsk"] = np.ascontiguousarray(
            eye4[ohc.reshape(NCH, 128, N_OH) & 3].transpose(1, 2, 0, 3)
        )  # (128, 8, 16, 4)
        mq = mhc.reshape(NCH, 128, N_MH, HIST) & 3  # (16, 128, 4, 20)
        m["mhmsk"] = np.ascontiguousarray(
            eye4[mq].transpose(1, 2, 0, 3, 4).reshape(128, N_MH, NCH * HIST, 4)
        )
        in_maps.append(m)
    return in_maps, c_consts, sig_bias


def _run(inputs, trace=False):
    from concourse.bass_utils import run_bass_kernel_spmd

    in_maps, c_consts, sig_bias = _prep_inputs(**inputs)
    nc = _build_program(c_consts, sig_bias)
    res = run_bass_kernel_spmd(nc, in_maps, core_ids=list(range(N_CORES)), trace=trace)
    outs = [
        res.results[c]["out"].reshape(128, NCH).T.reshape(BL) for c in range(N_CORES)
    ]
    full = np.concatenate(outs).reshape(B, 1).astype(np.float32)
    return full, res


def kernel(**inputs):
    full, _ = _run(inputs, trace=False)
    return full
